# revision 1
# baseline (speedup 1.0000x reference)
"""Trainium2 Bass kernel for nn_Conv2d_NN_spatial (retrieval_knn).

Full-input contract: kernel(**inputs) takes the unsharded inputs and returns
the full output. Internally: data-parallel over batch across 8 NeuronCores
(4 batches per core).

Per-core algorithm (per batch):
  1. negd2 = 2*dot(x2, xs) - m2  via one 65-row-contraction matmul per
     128-token chunk (n2 term omitted: constant per token, rank-invariant).
  2. top-3 neighbors per token via DVE max (top-8) + max_index
     (tie-break == jax top_k: first-unused occurrence, ascending).
  3. Projected tables P_k = W_k @ xs + bias/3 (one small matmul per k), laid
     out as [P0|P1|P2] so a single GPSIMD ap_gather with k-offset indices
     gathers all 3 neighbors; batch pairs stacked on partition halves.
  4. Sum the 3 gathered projections (2 DVE adds) -> output.
Pixel unshuffle/shuffle are pure layout transforms done host-side.
"""
import numpy as np

import concourse.bacc as bacc
import concourse.bass as bass
import concourse.mybir as mybir
import concourse.tile as tile
from concourse.bass_utils import run_bass_kernel_spmd

F32 = mybir.dt.float32
U16 = mybir.dt.uint16
I16 = mybir.dt.int16

N_CORES = 8
B_PER_CORE = 4
N_PAIRS = 2
C1 = 64          # unshuffled channels
N = 4096         # tokens per batch (64*64)
M = 256          # samples
K = 3
NCHUNK = 32      # 4096 / 128

SIDX = [0, 4, 8, 13, 17, 21, 25, 29, 34, 38, 42, 46, 50, 55, 59, 63]
SAMPLE_FLAT = (np.array(SIDX)[:, None] * 64 + np.array(SIDX)[None, :]).reshape(-1)

_CACHE = {}


def build_program():
    """Build the per-core Bass program (SPMD: same program on all 8 cores)."""
    nc = bacc.Bacc("TRN2", target_bir_lowering=False, debug=False,
                   enable_asserts=False)

    x2e = nc.dram_tensor("x2e", [B_PER_CORE, 65, N], F32, kind="ExternalInput")
    xse = nc.dram_tensor("xse", [B_PER_CORE, 66, M], F32, kind="ExternalInput")
    wke = nc.dram_tensor("wke", [K, 66, 64], F32, kind="ExternalInput")
    outd = nc.dram_tensor("outd", [N_PAIRS, 128, N], F32, kind="ExternalOutput")
    idxscr = nc.dram_tensor("idxscr", [B_PER_CORE, 128, 96], U16, kind="Internal")

    AA = mybir.AluOpType

    with tile.TileContext(nc) as tc:
        with (
            tc.tile_pool(name="xp", bufs=2) as xp,
            tc.tile_pool(name="sp", bufs=4) as sp,
            tc.tile_pool(name="cst", bufs=1) as cst,
            tc.tile_pool(name="tabp", bufs=2) as tabp,
            tc.tile_pool(name="dp", bufs=4) as dp,
            tc.tile_pool(name="m8p", bufs=4) as m8p,
            tc.tile_pool(name="ixp", bufs=2) as ixp,
            tc.tile_pool(name="wxp", bufs=2) as wxp,
            tc.tile_pool(name="gp", bufs=1) as gp,
            tc.tile_pool(name="op", bufs=2) as op_,
            tc.tile_pool(name="ps", bufs=4, space=bass.MemorySpace.PSUM) as psp,
            tc.tile_pool(name="ps2", bufs=2, space=bass.MemorySpace.PSUM) as psp2,
        ):
            onescol = cst.tile([64, 1], F32, tag="ones")
            nc.vector.memset(onescol[:], 1.0)
            wk = []
            for k in range(K):
                t = cst.tile([66, 64], F32, tag=f"wk{k}")
                nc.sync.dma_start(t[:], wke[k])
                wk.append(t)

            for pr in range(N_PAIRS):
                S_tiles = []
                X_tiles = []
                for half in range(2):
                    q = pr * 2 + half
                    X = xp.tile([65, N], F32, tag="X")
                    nc.sync.dma_start(X[:], x2e[q])
                    S = sp.tile([66, M], F32, tag="S")
                    nc.sync.dma_start(S[:], xse[q])
                    # m2 = sum_c (2 xs)^2 ; row 64 of S <- -m2/4... (scale -0.25)
                    SQ = sp.tile([64, M], F32, tag="SQ")
                    nc.vector.tensor_tensor(SQ[:], S[0:64, :], S[0:64, :], op=AA.mult)
                    m2ps = psp2.tile([128, M], F32, tag="m2ps")
                    nc.tensor.matmul(m2ps[64:65, :], onescol[:], SQ[:],
                                     tile_position=(0, 64))
                    nc.scalar.activation(S[64:65, :], m2ps[64:65, :],
                                         mybir.ActivationFunctionType.Identity,
                                         bias=0.0, scale=-0.25)
                    S_tiles.append(S)
                    X_tiles.append(X)

                # tables: [P0|P1|P2] (+bias/3 folded in), batch half on
                # partition half
                TAB = tabp.tile([128, K * M], F32, tag="TAB")
                for k in range(K):
                    tp = psp2.tile([128, M], F32, tag="tabps")
                    nc.tensor.matmul(tp[0:64, :], wk[k][:], S_tiles[0][:])
                    nc.tensor.matmul(tp[64:128, :], wk[k][:], S_tiles[1][:],
                                     tile_position=(0, 64))
                    nc.scalar.copy(TAB[:, k * M:(k + 1) * M], tp[:])

                WIDX = wxp.tile([128, 768], I16, tag="WIDX")
                for half in range(2):
                    q = pr * 2 + half
                    X, S = X_tiles[half], S_tiles[half]
                    IDXS = ixp.tile([128, 256], U16, tag="IDXS")
                    for c in range(NCHUNK):
                        nd = psp.tile([128, M], F32, tag="nd")
                        nc.tensor.matmul(nd[:], X[:, c * 128:(c + 1) * 128],
                                         S[0:65, :])
                        D = dp.tile([128, M], F32, tag="D")
                        nc.scalar.copy(D[:], nd[:])
                        M8 = m8p.tile([128, 8], F32, tag="M8")
                        nc.vector.max(M8[:], D[:])
                        nc.vector.max_index(IDXS[:, c * 8:(c + 1) * 8], M8[:], D[:])

                    # slice k<3 of each chunk's 8, add 256*k table offset
                    IDXC = ixp.tile([128, 96], U16, tag="IDXC")
                    src = IDXS[:].rearrange("p (c e) -> p c e", e=8)
                    dst = IDXC[:].rearrange("p (c e) -> p c e", e=3)
                    for k in range(K):
                        nc.vector.tensor_scalar_add(dst[:, :, k:k + 1],
                                                    src[:, :, k:k + 1], 256 * k)
                    # fold to wrapped gather-index layout via DRAM round-trip
                    nc.sync.dma_start(idxscr[q], IDXC[:])
                    folded = idxscr[q].rearrange("(th p) f -> p th f", p=16)
                    for g in range(4):
                        base = 64 * half + 16 * g
                        dst = WIDX[base:base + 16, :].bitcast(U16).rearrange(
                            "p (th f) -> p th f", th=8)
                        nc.sync.dma_start(dst, folded)

                G = gp.tile([128, 12288], F32, tag="G")
                nc.gpsimd.ap_gather(G[:], TAB[:], WIDX[:], channels=128,
                                    num_elems=768, d=1, num_idxs=12288)

                gv = G[:].rearrange("p (th c k w) -> p th c k w",
                                    th=8, c=32, k=K, w=16)
                T1 = op_.tile([128, N], F32, tag="T1")
                t1v = T1[:].rearrange("p (th c w) -> p th c w", th=8, c=32, w=16)
                OUT = op_.tile([128, N], F32, tag="OUT")
                ov = OUT[:].rearrange("p (th c w) -> p th c w", th=8, c=32, w=16)
                nc.vector.tensor_tensor(t1v, gv[:, :, :, 0, :],
                                        gv[:, :, :, 1, :], op=AA.add)
                nc.vector.tensor_tensor(ov, t1v, gv[:, :, :, 2, :], op=AA.add)
                nc.sync.dma_start(outd[pr], OUT[:])

    nc.compile()
    return nc


def host_prep(x, weight, bias):
    """Full inputs -> per-core in_maps (list of 8 dicts)."""
    x = np.ascontiguousarray(np.asarray(x), dtype=np.float32)
    weight = np.asarray(weight, dtype=np.float32)
    bias = np.asarray(bias, dtype=np.float32)
    B = x.shape[0]
    x1 = x.reshape(B, 16, 64, 2, 64, 2).transpose(0, 1, 3, 5, 2, 4)
    x2 = np.ascontiguousarray(x1).reshape(B, C1, N)
    xs = np.ascontiguousarray(x2[:, :, SAMPLE_FLAT])

    x2e = np.empty((B, 65, N), np.float32)
    x2e[:, :64] = x2
    x2e[:, 64] = 1.0
    xse = np.zeros((B, 66, M), np.float32)
    xse[:, :64] = xs * np.float32(2.0)
    xse[:, 65] = 1.0
    wke = np.zeros((K, 66, 64), np.float32)
    for k in range(K):
        wke[k, :64] = weight[:, :, k].T * np.float32(0.5)
        wke[k, 65] = bias * np.float32(1.0 / 3.0)

    in_maps = []
    for core in range(N_CORES):
        sl = slice(core * B_PER_CORE, (core + 1) * B_PER_CORE)
        in_maps.append({
            "x2e": np.ascontiguousarray(x2e[sl]),
            "xse": np.ascontiguousarray(xse[sl]),
            "wke": wke,
        })
    return in_maps


def host_post(results):
    """Per-core outd [2, 128, 4096] -> full output [32, 16, 128, 128]."""
    B = N_CORES * B_PER_CORE
    out = np.empty((B, C1, N), np.float32)
    for core in range(N_CORES):
        o = results[core]["outd"]  # [2, 128, 4096]
        for pr in range(N_PAIRS):
            for half in range(2):
                b = core * B_PER_CORE + pr * 2 + half
                dev = o[pr, 64 * half:64 * half + 64]  # [64, 4096] (th, c, p)
                out[b] = (dev.reshape(C1, 8, 32, 16)
                          .transpose(0, 2, 1, 3).reshape(C1, N))
    out = out.reshape(B, C1, 64, 64)
    y = (out.reshape(B, 16, 2, 2, 64, 64).transpose(0, 1, 4, 2, 5, 3)
         .reshape(B, 16, 128, 128))
    return np.ascontiguousarray(y)


def kernel(x, weight, bias):
    if "nc" not in _CACHE:
        _CACHE["nc"] = build_program()
    nc = _CACHE["nc"]
    in_maps = host_prep(x, weight, bias)
    res = run_bass_kernel_spmd(nc, in_maps, core_ids=list(range(N_CORES)))
    return host_post(res.results)

